# revision 1
# baseline (speedup 1.0000x reference)
"""MiniMoE Trainium2 kernel (expert-parallel, F-split across core pairs).

Problem (hardcoded): x [4, 2048, 1024] f32, router_w [1024, 4], router_b [4],
w1/w3 [4, 1024, 4096], w2 [4, 4096, 1024], top-2 of 4 experts, SwiGLU.

Strategy
--------
Host computes the (tiny) router + top-2 dispatch — this *is* the sharding
decision ("all-to-all token dispatch by top_indices"). Core pair (2e, 2e+1)
owns expert e: core 2e computes the F in [0, 2048) half of the SwiGLU FFN,
core 2e+1 the F in [2048, 4096) half, each over ALL tokens routed to expert
e. The two partial outputs sum to the expert output (h @ w2 is linear in h),
and the host scatter-adds them back with the renormalized gate weights.

On-device layout keeps features on partitions and tokens on the free axis
(so no transposes are needed between the two matmuls):
  hT[f, t]   = silu(w1.T @ xT) * (w3.T @ xT)      f on partitions
  outT[d, t] = w2.T @ hT                           d on partitions
All weights for the core's F-half stay resident in SBUF (~12 MB bf16);
tokens stream through in blocks of 512.
"""

import numpy as np
import ml_dtypes

import concourse.bass as bass
import concourse.bacc as bacc
import concourse.tile as tile
import concourse.mybir as mybir
from concourse.bass_utils import run_bass_kernel_spmd

B, S, D, F, E, TOPK = 4, 2048, 1024, 4096, 4, 2
N_CORES = 8
FH = F // 2          # F-half handled per core
P = 128              # SBUF partitions
ND = D // P          # 8 d-blocks
NF = FH // P         # 16 f-blocks per core
BF16 = mybir.dt.bfloat16
F32 = mybir.dt.float32

_NC_CACHE: dict[int, object] = {}


def _token_blocks(C: int) -> list[tuple[int, int]]:
    """Token blocks of 512, but split a short tail across the last two
    blocks (e.g. 512+128 -> 320+320): matmul N=320 pipelines against the
    128-cycle weight loads far better than N=128 does."""
    sizes = []
    left = C
    while left > 0:
        tb = min(512, left)
        sizes.append(tb)
        left -= tb
    if len(sizes) >= 2 and sizes[-1] < 512:
        pair = sizes[-2] + sizes[-1]
        hi = ((pair // 2 + 63) // 64) * 64
        sizes[-2:] = [hi, pair - hi]
    blocks, t0 = [], 0
    for tb in sizes:
        blocks.append((t0, tb))
        t0 += tb
    return blocks


def _build_nc(C: int, repeat: int = 1, ft_chunks: bool = True):
    """Build + compile the SPMD per-core program for capacity C tokens.

    repeat > 1 re-runs the whole token loop (timing harness use only —
    lets test.py fit out the fixed dispatch overhead via the slope).
    ft_chunks: load w1/w3 as 16 host-pre-tiled f-tile chunks (w1t/w3t
    inputs) instead of 8 d-row chunks of the plain [D, FH] layout."""
    nc = bacc.Bacc("TRN2", target_bir_lowering=False, debug=False,
                   num_devices=N_CORES)
    xT = nc.dram_tensor("xT", [D, C], BF16, kind="ExternalInput").ap()
    if ft_chunks:
        # Host-pre-tiled [NF, P, ND*P]: chunk ft is exactly the SBUF tile
        # for f-tile ft, so each chunk loads as one contiguous DMA.
        w1 = nc.dram_tensor("w1t", [NF, P, ND * P], BF16, kind="ExternalInput").ap()
        w3 = nc.dram_tensor("w3t", [NF, P, ND * P], BF16, kind="ExternalInput").ap()
    else:
        w1 = nc.dram_tensor("w1", [D, FH], BF16, kind="ExternalInput").ap()
        w3 = nc.dram_tensor("w3", [D, FH], BF16, kind="ExternalInput").ap()
    w2 = nc.dram_tensor("w2", [FH, D], BF16, kind="ExternalInput").ap()
    outT = nc.dram_tensor("outT", [D, C], F32, kind="ExternalOutput").ap()

    with tile.TileContext(nc) as tc:
        with (
            tc.tile_pool(name="wpool", bufs=1) as wpool,
            tc.tile_pool(name="xpool", bufs=3) as xpool,
            tc.tile_pool(name="hpool", bufs=3) as hpool,
            tc.tile_pool(name="tpool", bufs=3) as tpool,
            tc.tile_pool(name="opool", bufs=3) as opool,
            tc.tile_pool(name="ps1", bufs=2, space=bass.MemorySpace.PSUM) as ps1,
            tc.tile_pool(name="ps2", bufs=4, space=bass.MemorySpace.PSUM) as ps2,
        ):
            # Resident weights: partitions hold the contraction dim slice.
            # w1/w3 are chunked BY F-TILE (the phase-1 consumption order):
            # the first f-group only waits for ~0.5 MB instead of the whole
            # 16 MB, and demand then ramps at ~0.15 MB/us < DMA rate.
            w2_r = w2.rearrange("(n p) d -> n p d", p=P)
            blocks = _token_blocks(C) * repeat
            xT_r = xT.rearrange("(n p) c -> p n c", p=P)

            # Startup ordering: the first matmul needs w1[ft=0] plus the d=0
            # x-chunk of block 0 — emit exactly those first (DMA emission
            # order is queue order), then the rest of block-0 x per d-chunk,
            # then the remaining 24 MB weight stream.
            t0_first, TB_first = blocks[0]
            x0_d = []
            w1_f, w3_f, w2_f = [], [], []
            if ft_chunks:
                t1 = wpool.tile([P, ND, P], BF16, tag="w1_0")
                t3 = wpool.tile([P, ND, P], BF16, tag="w3_0")
                nc.sync.dma_start(t1[:], w1[0].rearrange("p (n c) -> p n c", c=P))
                nc.sync.dma_start(t3[:], w3[0].rearrange("p (n c) -> p n c", c=P))
                w1_f.append(t1)
                w3_f.append(t3)
                for d in range(ND):
                    xd = wpool.tile([P, TB_first], BF16, tag=f"x0_{d}")
                    nc.sync.dma_start(xd[:], xT_r[:, d, t0_first:t0_first + TB_first])
                    x0_d.append(xd)
                for ft in range(1, NF):
                    t1 = wpool.tile([P, ND, P], BF16, tag=f"w1_{ft}")
                    t3 = wpool.tile([P, ND, P], BF16, tag=f"w3_{ft}")
                    nc.sync.dma_start(t1[:], w1[ft].rearrange("p (n c) -> p n c", c=P))
                    nc.sync.dma_start(t3[:], w3[ft].rearrange("p (n c) -> p n c", c=P))
                    w1_f.append(t1)
                    w3_f.append(t3)
            else:
                w1_r = w1.rearrange("(n p) f -> n p f", p=P)
                w3_r = w3.rearrange("(n p) f -> n p f", p=P)
                w1_d, w3_d = [], []
                for d in range(ND):
                    t1 = wpool.tile([P, FH], BF16, tag=f"w1d_{d}")
                    t3 = wpool.tile([P, FH], BF16, tag=f"w3d_{d}")
                    nc.sync.dma_start(t1[:], w1_r[d])
                    nc.sync.dma_start(t3[:], w3_r[d])
                    w1_d.append(t1)
                    w3_d.append(t3)

                class _DView:
                    def __init__(self, tiles):
                        self.tiles = tiles

                    def __getitem__(self, ft):
                        return _FtView(self.tiles, ft)

                class _FtView:
                    def __init__(self, tiles, ft):
                        self.tiles, self.ft = tiles, ft

                    def __getitem__(self, key):
                        _, d, _ = key
                        f = self.ft
                        return self.tiles[d][:, f * P:(f + 1) * P]

                w1_f = _DView(w1_d)
                w3_f = _DView(w3_d)
            for ft in range(NF):
                t2 = wpool.tile([P, D], BF16, tag=f"w2_{ft}")
                nc.sync.dma_start(t2[:], w2_r[ft])
                w2_f.append(t2)

            for bi, (t0, TB) in enumerate(blocks):
                if bi == 0 and x0_d:
                    def xv(d):
                        return x0_d[d][:]
                else:
                    xtb = xpool.tile([P, ND, TB], BF16, tag="xtb")
                    nc.sync.dma_start(xtb[:], xT_r[:, :, t0:t0 + TB])

                    def xv(d, _x=xtb):
                        return _x[:, d, :]

                hT = hpool.tile([P, NF, TB], BF16, tag="hT")
                for ft in range(NF):
                    p1 = ps1.tile([P, TB], F32, tag="p1")
                    p3 = ps1.tile([P, TB], F32, tag="p3")
                    for d in range(ND):
                        nc.tensor.matmul(
                            p1[:], w1_f[ft][:, d, :],
                            xv(d), start=(d == 0), stop=(d == ND - 1))
                    for d in range(ND):
                        nc.tensor.matmul(
                            p3[:], w3_f[ft][:, d, :],
                            xv(d), start=(d == 0), stop=(d == ND - 1))
                    sil = tpool.tile([P, TB], F32, tag="sil")
                    nc.scalar.activation(
                        sil[:], p1[:], mybir.ActivationFunctionType.Silu)
                    nc.vector.tensor_mul(hT[:, ft, :], sil[:], p3[:])

                for db in range(ND):
                    po = ps2.tile([P, TB], F32, tag="po")
                    for ft in range(NF):
                        nc.tensor.matmul(
                            po[:], w2_f[ft][:, db * P:(db + 1) * P],
                            hT[:, ft, :], start=(ft == 0), stop=(ft == NF - 1))
                    ot = opool.tile([P, TB], F32, tag="ot")
                    nc.scalar.copy(ot[:], po[:])
                    nc.sync.dma_start(outT[db * P:(db + 1) * P, t0:t0 + TB], ot[:])

    nc.compile()
    return nc


def _route(x, router_w, router_b):
    """Host router: top-2 expert ids + renormalized gates (float64 math)."""
    T = x.shape[0] * x.shape[1]
    xf = x.reshape(T, D).astype(np.float64)
    logits = xf @ router_w.astype(np.float64) + router_b.astype(np.float64)
    # stable sort: ties resolve to the lowest expert id, like jax.lax.top_k
    order = np.argsort(-logits, axis=-1, kind="stable")   # [T, E] descending
    top_i = order[:, :TOPK]                        # [T, 2]
    top_l = np.take_along_axis(logits, top_i, axis=-1)
    top_l -= top_l.max(axis=-1, keepdims=True)
    ex = np.exp(top_l)
    gates = ex / ex.sum(axis=-1, keepdims=True)    # [T, 2] renormalized
    return top_i, gates


def prepare(x, router_w, router_b, w1, w3, w2):
    """Route on host, build per-core input maps. Returns (C, in_maps, meta)."""
    T = x.shape[0] * x.shape[1]
    xf = np.ascontiguousarray(x.reshape(T, D), dtype=np.float32)
    top_i, gates = _route(x, router_w, router_b)

    idx_per_e = []
    gate_per_e = []
    for e in range(E):
        mask = (top_i == e)
        rows = np.nonzero(mask.any(axis=-1))[0]
        g = np.where(mask[rows, 0], gates[rows, 0], gates[rows, 1])
        idx_per_e.append(rows)
        gate_per_e.append(g.astype(np.float32))

    # Only partition dims need 128-alignment; the token (free) dim doesn't,
    # so capacity is exactly the largest expert's token count.
    C = max(max(len(r) for r in idx_per_e), 1)

    in_maps = []
    for core in range(N_CORES):
        e, half = core // 2, core % 2
        fs = slice(half * FH, (half + 1) * FH)
        rows = idx_per_e[e]
        xg = np.zeros((C, D), np.float32)
        xg[:len(rows)] = xf[rows]

        def tile_w(w):  # [D, FH] -> [NF, P, ND*P], chunk ft == SBUF tile ft
            return np.ascontiguousarray(
                w.reshape(ND, P, NF, P).transpose(2, 1, 0, 3).reshape(NF, P, ND * P))

        w1e = w1[e, :, fs].astype(ml_dtypes.bfloat16)
        w3e = w3[e, :, fs].astype(ml_dtypes.bfloat16)
        in_maps.append({
            "xT": np.ascontiguousarray(xg.T).astype(ml_dtypes.bfloat16),
            "w1": np.ascontiguousarray(w1e),
            "w3": np.ascontiguousarray(w3e),
            "w1t": tile_w(w1e),
            "w3t": tile_w(w3e),
            "w2": np.ascontiguousarray(w2[e, fs, :]).astype(ml_dtypes.bfloat16),
        })
    meta = (T, idx_per_e, gate_per_e)
    return C, in_maps, meta


def combine(results, meta):
    """Gate-weighted scatter-add of the per-core partial expert outputs."""
    T, idx_per_e, gate_per_e = meta
    out = np.zeros((T, D), np.float32)
    for e in range(E):
        rows = idx_per_e[e]
        n = len(rows)
        part = (results[2 * e]["outT"].T[:n].astype(np.float32)
                + results[2 * e + 1]["outT"].T[:n].astype(np.float32))
        out[rows] += gate_per_e[e][:, None] * part
    return out.reshape(B, S, D)


def kernel(**inputs):
    x = np.asarray(inputs["x"], np.float32)
    router_w = np.asarray(inputs["router_w"], np.float32)
    router_b = np.asarray(inputs["router_b"], np.float32)
    w1 = np.asarray(inputs["w1"], np.float32)
    w3 = np.asarray(inputs["w3"], np.float32)
    w2 = np.asarray(inputs["w2"], np.float32)

    C, in_maps, meta = prepare(x, router_w, router_b, w1, w3, w2)
    if C not in _NC_CACHE:
        _NC_CACHE[C] = _build_nc(C)
    nc = _NC_CACHE[C]
    # prepare() emits both weight layouts (for A/B builds); pass the program
    # exactly its declared inputs — the native runner rejects extra keys.
    needed = {
        a.memorylocations[0].name
        for a in nc.m.functions[0].allocations
        if isinstance(a, mybir.MemoryLocationSet) and a.kind == "ExternalInput"
    }
    in_maps = [{k: v for k, v in m.items() if k in needed} for m in in_maps]
    res = run_bass_kernel_spmd(nc, in_maps, list(range(N_CORES)))
    return combine(res.results, meta)



# revision 19
# speedup vs baseline: 1.0469x; 1.0469x over previous
"""MiniMoE Trainium2 kernel — F/8-sliced expert weights + one-level Strassen.

Problem (hardcoded): x [4, 2048, 1024] f32, router_w [1024, 4], router_b [4],
w1/w3 [4, 1024, 4096], w2 [4, 4096, 1024], top-2 of 4 experts, SwiGLU.

Strategy
--------
Host computes the (tiny) router + top-2 dispatch. Every core processes ALL
(token, expert) pairs — the per-expert gathered token stream, concatenated
over experts (S = sum of padded per-expert counts, ~16.4k slots) — but owns
only a 512-wide slice of the FFN hidden dim (F/8 per core). The 8 partial
outputs (each summing that core's F-slice contribution over the full model
dim D) are added on the host and scatter-added into tokens with the gate
weights. This balances compute EXACTLY across cores for any routing (the
old expert-paired layout paid max_e(C_e)/mean_e(C_e) ~ 1.5% padding).

Per (expert-segment, 512-token block), with f = core's 512 F-slice:
  phase 1:  a = w1_e[:, f].T @ x   and   b = w3_e[:, f].T @ x,
            each [512, TB], via ONE LEVEL OF STRASSEN over
            (f: 2x256, d: 2x512, t: 2xTB/2): 7 products instead of 8 —
            7/8 of the tensor-engine cycles. The weight-side combos
            (A11+A22 etc.) are precomputed for free on the host; the
            x-side combos run on the otherwise-idle vector engine; the
            quadrant recombination chains run on DVE + scalar engines,
            all far below the PE's per-block cycle budget.
  phase 2:  outT[d, t] += w2_e[f, :].T @ (silu(a) * b)    (direct GEMM)

PSUM: 7 product tiles [128, 2, 256] f32 = 7 banks; phase-2 accumulators
double-buffer in the two half-banks of the LAST-computed product's bank
(M5), which is always free by then. Weight sets ping-pong between two SBUF
slots across expert segments (9 MB resident instead of 18 MB).

Numerics (measured on the seed-0 inputs, host simulation): plain bf16
2-expert MoE = 4.1e-3 rel err; + Strassen on w1/w3 = 7.5e-3; bf16 output
adds ~2e-3 in quadrature. Gate is 2e-2.
"""

import numpy as np
import ml_dtypes

import concourse.bass as bass
import concourse.bacc as bacc
import concourse.tile as tile
import concourse.mybir as mybir
from concourse.bass_utils import run_bass_kernel_spmd

B, S, D, F, E, TOPK = 4, 2048, 1024, 4096, 4, 2
N_CORES = 8
FSLICE = F // N_CORES   # 512 hidden features per core
FH = F // 2             # kept for test.py compat (old roofline print)
P = 128                 # SBUF partitions
ND = D // P             # 8 d-chunks
NFC = FSLICE // P       # 4 f-chunks per core
BF16 = mybir.dt.bfloat16
FP16 = mybir.dt.float16
F32 = mybir.dt.float32

# Strassen products in emission order. Each entry:
#   (A-combo builder over quadrants of A=[f 512, d 1024], rhs spec)
# rhs spec: ("xc", j) = DVE-built x combo j, ("x", dhalf, thalf) = view.
# A quadrants: A11=[:256,:512] A12=[:256,512:] A21=[256:,:512] A22=[256:,512:]
# B quadrants: B11=x[:512,:T2] B12=x[:512,T2:] B21=x[512:,:T2] B22=x[512:,T2:]
#   M6=(A21-A11)(B11+B12)  M7=(A12-A22)(B21+B22)  M2=(A21+A22)B11
#   M3=A11(B12-B22)        M1=(A11+A22)(B11+B22)  M4=A22(B21-B11)
#   M5=(A11+A12)B22
# C11=M1+M4-M5+M7  C12=M3+M5  C21=M2+M4  C22=M1-M2+M3+M6
PRODUCTS = ["m6", "m7", "m2", "m3", "m1", "m4", "m5"]
# x-combos (DVE): j -> (lhs dhalf, lhs thalf, rhs dhalf, rhs thalf, op)
XCOMBOS = {
    "xc4": (0, 0, 0, 1, "add"),   # B11+B12  (for M6)
    "xc5": (1, 0, 1, 1, "add"),   # B21+B22  (for M7)
    "xc3": (0, 1, 1, 1, "sub"),   # B12-B22  (for M3)
    "xc1": (0, 0, 1, 1, "add"),   # B11+B22  (for M1)
    "xc2": (1, 0, 0, 0, "sub"),   # B21-B11  (for M4)
}
PROD_RHS = {
    "m6": ("xc", "xc4"), "m7": ("xc", "xc5"), "m2": ("x", 0, 0),
    "m3": ("xc", "xc3"), "m1": ("xc", "xc1"), "m4": ("xc", "xc2"),
    "m5": ("x", 1, 1),
}

_NC_CACHE: dict[int, object] = {}


def _token_blocks(C: int) -> list[tuple[int, int]]:
    """Token blocks of 1024 with the short tail split across the last two
    blocks (e.g. 1024+24 -> 576+472); C must be a multiple of 4 so every
    block's Strassen t-half stays even."""
    sizes = []
    left = C
    while left > 0:
        tb = min(1024, left)
        sizes.append(tb)
        left -= tb
    if len(sizes) >= 2 and sizes[-1] < 1024:
        pair = sizes[-2] + sizes[-1]
        hi = ((pair // 2 + 63) // 64) * 64
        sizes[-2:] = [hi, pair - hi]
    blocks, t0 = [], 0
    for tb in sizes:
        blocks.append((t0, tb))
        t0 += tb
    return blocks


def _build_nc(seg_key, repeat: int = 1):
    """Build + compile the SPMD per-core program.

    seg_key: tuple of per-expert (even) padded token counts. repeat > 1
    re-runs the whole segment loop (timing-harness use only)."""
    seg_lens = list(seg_key)
    S_tot = sum(seg_lens)
    nc = bacc.Bacc("TRN2", target_bir_lowering=False, debug=False,
                   num_devices=N_CORES)
    xT = nc.dram_tensor("xT", [D, S_tot], FP16, kind="ExternalInput").ap()
    # per-expert Strassen weight combos, packed [E, P, 7*ND2*256] where
    # ND2 = 4 d-chunks of the 512-contraction half; product i's chunk is
    # contiguous per partition.
    w1s = nc.dram_tensor("w1s", [E, P, 7 * 4 * 256], FP16,
                         kind="ExternalInput").ap()
    w3s = nc.dram_tensor("w3s", [E, P, 7 * 4 * 256], FP16,
                         kind="ExternalInput").ap()
    # w2 slice, lhsT layout [E, P(f within chunk), NFC*D]
    w2t = nc.dram_tensor("w2t", [E, P, NFC * D], FP16,
                         kind="ExternalInput").ap()
    outT = nc.dram_tensor("outT", [D, S_tot], FP16, kind="ExternalOutput").ap()

    xT_r = xT.rearrange("(n p) s -> p n s", p=P)
    outT_r = outT.rearrange("(n p) s -> n p s", p=P)

    with tile.TileContext(nc) as tc:
        with (
            tc.tile_pool(name="wpool", bufs=1) as wpool,
            tc.tile_pool(name="xpool", bufs=3) as xpool,
            tc.tile_pool(name="xcpool", bufs=1) as xcpool,
            tc.tile_pool(name="abpool", bufs=2) as abpool,
            tc.tile_pool(name="spool", bufs=1) as spool,
            tc.tile_pool(name="hpool", bufs=2) as hpool,
            tc.tile_pool(name="opool", bufs=2) as opool,
            tc.tile_pool(name="pspool", bufs=1,
                         space=bass.MemorySpace.PSUM) as pspool,
        ):
            _build_body(nc, tc, wpool, xpool, xcpool, abpool, spool, hpool,
                        opool, pspool, w1s, w3s, w2t, xT_r, outT_r,
                        seg_lens, repeat)

    nc.compile()
    return nc


def _build_body(nc, tc, wpool, xpool, xcpool, abpool, spool, hpool,
                opool, pspool, w1s, w3s, w2t, xT_r, outT_r,
                seg_lens, repeat):
    E_ = E
    segseq = [(rep, e) for rep in range(repeat) for e in range(E_)]

    def emit_ws(idx, x0_hook=None):
        """Emit the w1/w3 combo DMAs for segment-sequence entry idx into the
        ping-pong slot. x0_hook (startup only) is called after the first
        product's combos so block 0's x lands ahead of the 7 MB bulk. The w2
        slice is single-slot and emitted inside the segment's first block
        (it must follow the previous segment's last phase-2 emission)."""
        _, e = segseq[idx]
        slot = idx % 2
        t1 = wpool.tile([P, 7, 4, 256], FP16, tag=f"w1_{slot}")
        t3 = wpool.tile([P, 7, 4, 256], FP16, tag=f"w3_{slot}")
        src1 = w1s[e].rearrange("p (m c f) -> p m c f", m=7, c=4)
        src3 = w3s[e].rearrange("p (m c f) -> p m c f", m=7, c=4)
        # first-executed product leads the stream (startup block 0 runs m2
        # first — its rhs is a raw x view, so compute starts ~1.5 MB in)
        i0 = PRODUCTS.index("m2") if x0_hook is not None else 0
        nc.sync.dma_start(t1[:, i0], src1[:, i0])
        nc.sync.dma_start(t3[:, i0], src3[:, i0])
        x0 = x0_hook() if x0_hook is not None else None
        for i in range(7):
            if i == i0:
                continue
            nc.sync.dma_start(t1[:, i], src1[:, i])
            nc.sync.dma_start(t3[:, i], src3[:, i])
        return {"w1": t1, "w3": t3, "x0": x0}

    # phase 2 of block b is emitted AFTER phase 1 of block b+1: the PE
    # fills block b's DVE-recomb + silu + mul latency with block b+1's
    # product matmuls instead of idling.
    pending_p2 = None

    def emit_p2(hT, w2tile, g0, TB):
        T2 = TB // 2
        for db in range(ND):
            ot = opool.tile([P, TB], FP16, tag="ot")
            for th in range(2):
                # two independent full-bank accumulators so group g+1's
                # matmuls never wait on group g's PSUM->SBUF copy
                po_t = pspool.tile([P, 512], F32,
                                   tag=f"po{(db * 2 + th) % 2}")
                po = po_t[:, 0:T2]
                for fc in range(NFC):
                    nc.tensor.matmul(
                        po[:],
                        w2tile[:, fc, db * P:(db + 1) * P],
                        hT[:, fc, T2 * th:T2 * th + T2],
                        start=(fc == 0), stop=(fc == NFC - 1))
                nc.scalar.copy(ot[:, T2 * th:T2 * th + T2], po[:])
            nc.sync.dma_start(outT_r[db, :, g0:g0 + TB], ot[:])

    blocks0 = _token_blocks(seg_lens[segseq[0][1]])

    def x0_hook():
        T2 = blocks0[0][1] // 2
        pair = []
        for th in range(2):
            xt = xpool.tile([P, ND, T2], FP16, tag="xtb")
            nc.sync.dma_start(xt[:], xT_r[:, :, T2 * th:T2 * (th + 1)])
            pair.append(xt)
        return pair

    ws = emit_ws(0, x0_hook)
    ws_next = None
    nprod = 0  # rolling product index -> 3 rotating double-bank PSUM slots

    for idx, (rep, e) in enumerate(segseq):
        if idx > 0:
            ws = ws_next
        t_base = sum(seg_lens[:e])
        blocks = _token_blocks(seg_lens[e])
        for bi, (t0, TB) in enumerate(blocks):
            T2 = TB // 2
            g0 = t_base + t0
            if idx == 0 and bi == 0:
                xth = ws["x0"]
            else:
                xth = []
                for th in range(2):
                    xt = xpool.tile([P, ND, T2], FP16, tag="xtb")
                    nc.sync.dma_start(
                        xt[:], xT_r[:, :, g0 + T2 * th:g0 + T2 * (th + 1)])
                    xth.append(xt)

            def xq(dhalf, thalf, _x=xth):
                return _x[thalf][:, 4 * dhalf:4 * dhalf + 4, :]

            # x-side Strassen combos (shared by the w1 and w3 GEMMs)
            xc = {}
            for j, (ld, lt, rd, rt, op) in XCOMBOS.items():
                c = xcpool.tile([P, 4, T2], FP16, tag=j)
                if op == "add":
                    nc.vector.tensor_add(c[:], xq(ld, lt), xq(rd, rt))
                else:
                    nc.vector.tensor_sub(c[:], xq(ld, lt), xq(rd, rt))
                xc[j] = c

            ab = {}
            for gname in ("w1", "w3"):
                wt = ws[gname]
                a = abpool.tile([P, NFC, TB], FP16, tag=f"a_{gname}")
                m = {}
                order = PRODUCTS
                if idx == 0 and bi == 0:
                    order = ["m2", "m6", "m7", "m3", "m1", "m4", "m5"]
                for pname in order:
                    i = PRODUCTS.index(pname)
                    # fixed max shape + slice: varying shapes under one PSUM
                    # tag get shape-dependent addresses, and the pipelined
                    # phase-2 accumulator then aliases the next block's
                    # product tiles (first odd-sized block corrupts)
                    ps_t = pspool.tile([P, 2, 512], F32,
                                       tag=f"mm{nprod % 3}")
                    ps = ps_t[:, :, 0:T2]
                    nprod += 1
                    spec = PROD_RHS[pname]
                    if spec[0] == "xc":
                        rhs = xc[spec[1]]
                        rview = lambda c, _r=rhs: _r[:, c, :]
                    else:
                        _, dh, th = spec
                        rv = xq(dh, th)
                        rview = lambda c, _r=rv: _r[:, c, :]
                    for fc in range(2):
                        for c in range(4):
                            nc.tensor.matmul(
                                ps[:, fc, :],
                                wt[:, i, c, fc * P:(fc + 1) * P],
                                rview(c),
                                start=(c == 0), stop=(c == 3))
                    # evict the product to SBUF fp16 immediately: the
                    # single copy is the bank's only consumer (slot frees
                    # fast), and the recomb chains then run on all-16-bit
                    # SBUF operands = the DVE's fast path. Copies alternate
                    # ACT/DVE to split the load.
                    mc = spool.tile([P, 2, T2], FP16, tag=f"mc_{pname}")
                    if i % 2 == 0:
                        nc.scalar.copy(mc[:], ps[:])
                    else:
                        nc.vector.tensor_copy(mc[:], ps[:])
                    m[pname] = mc

                    # recombination chains, eager, SBUF fp16
                    if pname == "m6" and "m2" in m:  # startup order
                        s22b = spool.tile([P, 2, T2], FP16, tag="s22b")
                        nc.vector.tensor_sub(s22b[:], m["m6"][:], m["m2"][:])
                    elif pname == "m2" and "m6" in m:  # normal order
                        s22b = spool.tile([P, 2, T2], FP16, tag="s22b")
                        nc.vector.tensor_sub(s22b[:], m["m6"][:], m["m2"][:])
                    elif pname == "m3":
                        s22x = spool.tile([P, 2, T2], FP16, tag="s22x")
                        nc.vector.tensor_add(s22x[:], s22b[:], m["m3"][:])
                    elif pname == "m1":
                        # C22 = M6-M2+M3+M1 done -> a[:, 2:4, T2:]
                        nc.vector.tensor_add(
                            a[:, 2:4, T2:TB], s22x[:], m["m1"][:])
                        s11b = spool.tile([P, 2, T2], FP16, tag="s11b")
                        nc.vector.tensor_add(s11b[:], m["m7"][:], m["m1"][:])
                    elif pname == "m4":
                        # C21 = M2+M4 done -> a[:, 2:4, :T2]
                        nc.vector.tensor_add(
                            a[:, 2:4, 0:T2], m["m2"][:], m["m4"][:])
                        s11x = spool.tile([P, 2, T2], FP16, tag="s11x")
                        nc.vector.tensor_add(s11x[:], s11b[:], m["m4"][:])
                    elif pname == "m5":
                        # C11 = M7+M1+M4-M5 done -> a[:, 0:2, :T2]
                        nc.vector.tensor_sub(
                            a[:, 0:2, 0:T2], s11x[:], m["m5"][:])
                        # C12 = M3+M5 done -> a[:, 0:2, T2:]
                        nc.vector.tensor_add(
                            a[:, 0:2, T2:TB], m["m3"][:], m["m5"][:])
                ab[gname] = a

            sil = spool.tile([P, NFC, TB], FP16, tag="sil")
            nc.scalar.activation(
                sil[:], ab["w1"][:], mybir.ActivationFunctionType.Silu)
            hT = hpool.tile([P, NFC, TB], FP16, tag="hT")
            nc.vector.tensor_mul(hT[:], sil[:], ab["w3"][:])

            if pending_p2 is not None:
                pending_p2()
            if bi == 0:
                # single-slot w2: the DMA must follow the previous
                # segment's last phase-2 emission (it overwrites the slot)
                w2tile = wpool.tile([P, NFC, D], FP16, tag="w2")
                nc.sync.dma_start(
                    w2tile[:],
                    w2t[e].rearrange("p (c d) -> p c d", c=NFC))
                ws["w2"] = w2tile
            pending_p2 = (lambda _h=hT, _w2=ws["w2"], _g0=g0, _TB=TB:
                          emit_p2(_h, _w2, _g0, _TB))

            if bi == min(1, len(blocks) - 1) and idx + 1 < len(segseq):
                # prefetch the next segment's weight set: emitted after
                # block 1's x DMA so it streams behind it in the queue,
                # finishing long before the boundary
                ws_next = emit_ws(idx + 1)

    if pending_p2 is not None:
        pending_p2()


def _route(x, router_w, router_b):
    """Host router: top-2 expert ids + renormalized gates (float64 math)."""
    T = x.shape[0] * x.shape[1]
    xf = x.reshape(T, D).astype(np.float64)
    logits = xf @ router_w.astype(np.float64) + router_b.astype(np.float64)
    # stable sort: ties resolve to the lowest expert id, like jax.lax.top_k
    order = np.argsort(-logits, axis=-1, kind="stable")
    top_i = order[:, :TOPK]
    top_l = np.take_along_axis(logits, top_i, axis=-1)
    top_l -= top_l.max(axis=-1, keepdims=True)
    ex = np.exp(top_l)
    gates = ex / ex.sum(axis=-1, keepdims=True)
    return top_i, gates


def _strassen_pack(A):
    """A [512 f, 1024 d] f32 -> packed [P, 7*4*256] bf16 lhsT combos in
    PRODUCTS emission order (M6 M7 M2 M3 M1 M4 M5)."""
    A11, A12 = A[:256, :512], A[:256, 512:]
    A21, A22 = A[256:, :512], A[256:, 512:]
    combos = [A21 - A11, A12 - A22, A21 + A22, A11, A11 + A22, A22, A11 + A12]
    # lhsT for the PE: [d 512, f 256] per combo -> 4 chunks [128, 256]
    packed = np.stack([c.T.reshape(4, P, 256) for c in combos])  # [7,4,128,256]
    packed = packed.transpose(2, 0, 1, 3).reshape(P, 7 * 4 * 256)
    return np.ascontiguousarray(packed.astype(np.float16))


def prepare(x, router_w, router_b, w1, w3, w2):
    """Route on host, build per-core input maps. Returns (key, in_maps, meta)."""
    T = x.shape[0] * x.shape[1]
    xf = np.ascontiguousarray(x.reshape(T, D), dtype=np.float32)
    top_i, gates = _route(x, router_w, router_b)

    idx_per_e, gate_per_e, seg_lens = [], [], []
    for e in range(E):
        mask = (top_i == e)
        rows = np.nonzero(mask.any(axis=-1))[0]
        g = np.where(mask[rows, 0], gates[rows, 0], gates[rows, 1])
        idx_per_e.append(rows)
        gate_per_e.append(g.astype(np.float32))
        # multiple of 4: blocks and their Strassen t-halves stay even
        seg_lens.append(((len(rows) + 3) // 4) * 4)
    seg_key = tuple(seg_lens)
    S_tot = sum(seg_lens)

    xg = np.zeros((S_tot, D), np.float32)
    off = 0
    for e in range(E):
        xg[off:off + len(idx_per_e[e])] = xf[idx_per_e[e]]
        off += seg_lens[e]
    xT = np.ascontiguousarray(xg.T).astype(np.float16)

    in_maps = []
    for core in range(N_CORES):
        fs = slice(core * FSLICE, (core + 1) * FSLICE)
        w1s = np.stack([_strassen_pack(w1[e][:, fs].T.astype(np.float32))
                        for e in range(E)])
        w3s = np.stack([_strassen_pack(w3[e][:, fs].T.astype(np.float32))
                        for e in range(E)])
        # w2 lhsT: contract over f (512 -> 4 chunks of 128), out cols d
        w2p = np.stack([
            np.ascontiguousarray(
                w2[e][fs, :].reshape(NFC, P, D).transpose(1, 0, 2)
                .reshape(P, NFC * D).astype(np.float16))
            for e in range(E)])
        in_maps.append({"xT": xT, "w1s": w1s, "w3s": w3s, "w2t": w2p})
    meta = (T, idx_per_e, gate_per_e, seg_lens)
    return seg_key, in_maps, meta


def combine(results, meta):
    """Sum the 8 cores' F-slice partial outputs, then gate-weighted
    scatter-add into token order."""
    T, idx_per_e, gate_per_e, seg_lens = meta
    acc = results[0]["outT"].astype(np.float32)
    for r in results[1:]:
        acc += r["outT"].astype(np.float32)
    out = np.zeros((T, D), np.float32)
    off = 0
    for e in range(E):
        rows = idx_per_e[e]
        out[rows] += gate_per_e[e][:, None] * acc[:, off:off + len(rows)].T
        off += seg_lens[e]
    return out.reshape(B, S, D)


def kernel(**inputs):
    x = np.asarray(inputs["x"], np.float32)
    router_w = np.asarray(inputs["router_w"], np.float32)
    router_b = np.asarray(inputs["router_b"], np.float32)
    w1 = np.asarray(inputs["w1"], np.float32)
    w3 = np.asarray(inputs["w3"], np.float32)
    w2 = np.asarray(inputs["w2"], np.float32)

    seg_key, in_maps, meta = prepare(x, router_w, router_b, w1, w3, w2)
    if seg_key not in _NC_CACHE:
        _NC_CACHE[seg_key] = _build_nc(seg_key)
    nc = _NC_CACHE[seg_key]
    res = run_bass_kernel_spmd(nc, in_maps, list(range(N_CORES)))
    return combine(res.results, meta)
